# revision 1
# baseline (speedup 1.0000x reference)
"""Block-FFT circulant matmul (BlockFFTDirectPrior) as a Trainium2 Bass kernel.

Math: out = ifft( einsum('bjf,ijf->bif', fft(x_blocks), conj(W_full)) ).real
with 64x64 blocks of size 256, batch 2048.

All matmul-based (no FFT butterflies):
  stage 1: per input block j, spectrum = x_j @ R            (DFT as matmul)
  stage 2: per frequency slot s, mix blocks j -> i with a 128x128 real
           matrix G_s built from W_real/W_imag (re/im packed)
  stage 3: per output block i, time = spectrum_i @ Rinv     (IDFT as matmul)

v2 vs v1: the two inter-stage permutes (partition-axis rotations) are done
with DMA xbar transposes (out[p, f, l] = in[l, f*128 + p]) instead of PE
identity matmuls — this removes ~512 small matmuls + 64 PSUM copies per
core.  Passes are software-pipelined S1p0|S2p0|S1p1|S3p0|S2p1|S3p1 so the
PE never waits on a transpose: each xbar runs concurrently with a later
stage's matmuls.  Four 32KB big buffers are time-shared across roles
(out1/X2/O2/T2) between the two passes.  Output is stored bf16 and
upcast on host.

Sharding: data-parallel over batch across 8 NeuronCores (256 rows each),
2 passes of 128 rows per core.  All matmul operands are bf16.
"""

import numpy as np
import ml_dtypes

import concourse.bass as bass
import concourse.mybir as mybir
from concourse import bacc
from concourse.tile import TileContext
from concourse.bass_utils import run_bass_kernel_spmd

B, KIN, KOUT, BLOCK = 2048, 64, 64, 256
NCORES = 8
BC = B // NCORES            # 256 batch rows per core
NPASS = 2
PB = BC // NPASS            # 128 batch rows per pass

F32 = mybir.dt.float32
BF16 = mybir.dt.bfloat16
NPBF16 = ml_dtypes.bfloat16

_NC_CACHE = {}


def _build_consts():
    """DFT / inverse-DFT matrices, bf16, kernel layouts."""
    t = np.arange(BLOCK)
    f = np.arange(128)
    ang = 2.0 * np.pi * np.outer(t, f) / BLOCK          # [t, f]
    RA = np.cos(ang)                                    # re f=0..127
    RB = -np.sin(ang)                                   # im f=1..127
    RB[:, 0] = np.cos(np.pi * t)                        # re f=128 in col 0
    R = np.zeros((2, 2, 128, 128), dtype=NPBF16)        # [h, kt, t(128), s]
    for kt in range(2):
        R[0, kt] = RA[kt * 128:(kt + 1) * 128, :].astype(NPBF16)
        R[1, kt] = RB[kt * 128:(kt + 1) * 128, :].astype(NPBF16)

    s = np.arange(128)
    tp = np.arange(BLOCK)
    angi = 2.0 * np.pi * np.outer(s, tp) / BLOCK        # [s, t']
    w = np.full((128, 1), 2.0 / BLOCK)
    w[0] = 1.0 / BLOCK
    RiA = w * np.cos(angi)
    RiB = -(2.0 / BLOCK) * np.sin(angi)
    RiB[0, :] = (1.0 / BLOCK) * np.cos(np.pi * tp)      # Nyquist (real) term
    Ri = np.stack([RiA, RiB]).astype(NPBF16)            # [2, 128, 256]
    return R, Ri


def _build_g(Wr, Wi):
    """Stage-2 mixing matrices, layout [k=(h*64+j), s, m=(re_i|im_i)], bf16."""
    G = np.zeros((128, 128, 128), dtype=np.float32)     # [s, k, m]
    G[0, :64, :64] = Wr[:, :, 0].T
    G[0, 64:, 64:] = Wr[:, :, 128].T
    WrT = np.transpose(Wr, (2, 1, 0))                   # [f, j, i]
    WiT = np.transpose(Wi, (2, 1, 0))
    G[1:, :64, :64] = WrT[1:128]
    G[1:, :64, 64:] = -WiT[1:128]
    G[1:, 64:, :64] = WiT[1:128]
    G[1:, 64:, 64:] = WrT[1:128]
    return np.ascontiguousarray(G.transpose(1, 0, 2)).astype(NPBF16)


def _build_nc():
    nc = bacc.Bacc("TRN2", target_bir_lowering=False, debug=False)
    # xP layout [pass, t(256), b(128), j(64)]
    xP = nc.dram_tensor("xP", [NPASS, BLOCK, PB, KIN], BF16, kind="ExternalInput")
    Gt = nc.dram_tensor("G", [128, 128, 128], BF16, kind="ExternalInput")
    Rt = nc.dram_tensor("R", [2, 2, 128, 128], BF16, kind="ExternalInput")
    Rit = nc.dram_tensor("Ri", [2, 128, 256], BF16, kind="ExternalInput")
    Y = nc.dram_tensor("Y", [BC, KOUT * BLOCK], BF16, kind="ExternalOutput")

    nck = [0]

    def copy_eng():
        nck[0] += 1
        return nc.vector.tensor_copy if nck[0] % 2 == 0 else nc.scalar.copy

    with TileContext(nc) as tc:
        with (
            tc.tile_pool(name="const", bufs=1) as cpool,
            tc.tile_pool(name="big", bufs=1) as bigpool,
            tc.tile_pool(name="work", bufs=2) as wpool,
            tc.tile_pool(name="ps", bufs=4, space="PSUM") as pspool,
        ):
            Rsb = cpool.tile([128, 4 * 128], BF16)
            for h in range(2):
                for kt in range(2):
                    nc.sync.dma_start(
                        Rsb[:, (h * 2 + kt) * 128:(h * 2 + kt + 1) * 128],
                        Rt.ap()[h, kt],
                    )
            Risb = cpool.tile([128, 512], BF16)
            for h in range(2):
                nc.sync.dma_start(Risb[:, h * 256:(h + 1) * 256], Rit.ap()[h])
            Gsb = cpool.tile([128, 128 * 128], BF16)
            for q in range(8):
                nc.gpsimd.dma_start(
                    Gsb[:, q * 2048:(q + 1) * 2048],
                    Gt.ap()[:, q * 16:(q + 1) * 16],
                )

            # four time-shared 32KB big buffers
            bufs = [
                bigpool.tile([128, 128 * 128], BF16, tag=f"buf{i}",
                             name=f"buf{i}")
                for i in range(4)
            ]
            # role map per pass: out1, X2, O2, T2
            # BISECT: same distinct buffer per role both passes
            # T2(p1) aliases bufs[0]: out1 is dead once perm1(p1) has read
            # it (same sync queue, ~36us earlier), so perm2(p1) can write it
            # without waiting on S3(p0)'s reads of bufs[3].
            roles = [
                (bufs[0], bufs[1], bufs[2], bufs[3]),   # pass 0
                (bufs[0], bufs[1], bufs[2], bufs[0]),   # pass 1
            ]

            # xk: [t(128), kt, b(128), j(64)]; single buffer, reloaded per
            # pass.  Pass 0 loads on the (idle) scalar hwdge so stage 1 can
            # start ~15us earlier; transfers finish before the first xbar.
            def load_xk(p, eng=None):
                eng = eng or nc.sync
                xk = bigpool.tile([128, 2, PB, KIN], BF16, tag="xk", name="xk")
                for q in range(4):
                    for kt in range(2):
                        eng.dma_start(
                            xk[:, kt, q * 32:(q + 1) * 32, :],
                            xP.ap()[p, kt * 128:(kt + 1) * 128,
                                    q * 32:(q + 1) * 32],
                        )
                return xk

            def stage1(p, xk, nchunks=2):
                out1, X2 = roles[p][0], roles[p][1]
                out1v = out1.rearrange("p (b hj) -> p b hj", hj=128)
                X2v = X2.rearrange("p (b s) -> p b s", s=128)
                cstep = 8 // nchunks            # g2-groups per xbar chunk
                for g2 in range(8):             # 16-batch groups
                    for h in range(2):
                        ps = pspool.tile([128, 1024], F32, tag="ps")
                        for q in range(2):
                            g = g2 * 2 + q      # 8-batch chunk
                            nc.tensor.matmul(
                                ps[:, q * 512:(q + 1) * 512],
                                Rsb[:, (h * 2) * 128:(h * 2 + 1) * 128],
                                xk[:, 0, g * 8:(g + 1) * 8, :],
                                start=True, stop=False,
                            )
                            nc.tensor.matmul(
                                ps[:, q * 512:(q + 1) * 512],
                                Rsb[:, (h * 2 + 1) * 128:(h * 2 + 2) * 128],
                                xk[:, 1, g * 8:(g + 1) * 8, :],
                                start=False, stop=True,
                            )
                        # ps [s, (b16, j64)] -> out1 [s, b, h*64+j]
                        copy_eng()(
                            out1v[:, g2 * 16:(g2 + 1) * 16,
                                  h * 64:(h + 1) * 64],
                            ps.rearrange("p (b j) -> p b j", b=16),
                        )
                    # permute-1 in nchunks big xbars — few instructions
                    # keeps sem-slot pressure low
                    if g2 % cstep == cstep - 1:
                        c = g2 // cstep
                        nc.sync.dma_start(
                            X2v[:, c * cstep * 16:(c + 1) * cstep * 16, :],
                            out1[:, c * cstep * 2048:(c + 1) * cstep * 2048],
                            transpose=True,
                        )

            def stage2(p):
                X2, O2, T2 = roles[p][1], roles[p][2], roles[p][3]
                X2v = X2.rearrange("p (b s) -> p b s", s=128)
                O2v = O2.rearrange("p (b s) -> p b s", s=128)
                T2v = T2.rearrange("p (b m) -> p b m", m=128)
                for sg in range(16):            # 8 slots per PSUM tile
                    ps = pspool.tile([128, 1024], F32, tag="ps")
                    for u in range(8):
                        s = sg * 8 + u
                        nc.tensor.matmul(
                            ps[:, u * 128:(u + 1) * 128],
                            Gsb[:, s * 128:(s + 1) * 128],
                            X2v[:, :, s],
                            start=True, stop=True,
                        )
                    # ps [m, (s8, b)] -> O2 [m, b, s]  (strided read)
                    copy_eng()(
                        O2v[:, :, sg * 8:(sg + 1) * 8],
                        ps.rearrange("p (s b) -> p b s", s=8),
                    )
                # permute-2: O2 cols (b,s) -> T2 [s, b, m]; deferred so the
                # caller controls where the (sync-serialized) xbars go.
                def emit_perm2():
                    nc.sync.dma_start(
                        T2v[:, :, :], O2[:, :], transpose=True,
                    )
                return [emit_perm2]

            def stage3(p, g8s):
                T2 = roles[p][3]
                T2v = T2.rearrange("p (b m) -> p b m", m=128)
                for g8 in g8s:                  # 8 output blocks i per store
                    yt = wpool.tile([128, 2048], BF16, tag="yt", name="yt")
                    for half in range(2):
                        ps = pspool.tile([128, 1024], F32, tag="ps")
                        for q in range(4):
                            i = g8 * 8 + half * 4 + q
                            nc.tensor.matmul(
                                ps[:, q * 256:(q + 1) * 256],
                                T2v[:, :, i],
                                Risb[:, 0:256], start=True, stop=False,
                            )
                            nc.tensor.matmul(
                                ps[:, q * 256:(q + 1) * 256],
                                T2v[:, :, 64 + i],
                                Risb[:, 256:512], start=False, stop=True,
                            )
                        copy_eng()(
                            yt[:, half * 1024:(half + 1) * 1024], ps[:, :]
                        )
                    nc.gpsimd.dma_start(
                        Y.ap()[p * PB:(p + 1) * PB,
                               g8 * 2048:(g8 + 1) * 2048],
                        yt[:, :],
                    )

            # Pipelined emission.  All xbar transposes stay on the sync
            # engine (concurrent xbars from two queues corrupt each other on
            # HW); PE work is interleaved to cover every transpose tail:
            #   perm2(p0) runs under S1(p1); perm1(p1) tail under S3(p0)a;
            #   perm2(p1) runs under S3(p0)b.
            xk0 = load_xk(0)
            stage1(0, xk0)
            xk1 = load_xk(1)            # sync, runs during S2(0)
            p2_0 = stage2(0)
            for emit in p2_0:           # perm2(p0) en-bloc on sync
                emit()
            stage1(1, xk1, nchunks=1)   # PE covers perm2(p0)
            stage3(0, range(0, 5))      # PE covers perm1(p1) tail
            p2_1 = stage2(1)
            stage3(0, range(5, 8))      # S3(p0) tail: last readers of T2
            for emit in p2_1:           # perm2(p1): WAR after S3(p0) reads
                emit()
            stage3(1, range(8))
    nc.compile()
    return nc


def _get_nc():
    if "nc" not in _NC_CACHE:
        _NC_CACHE["nc"] = _build_nc()
    return _NC_CACHE["nc"]


def run(x, W_real, W_imag, trace=False):
    x = np.asarray(x, dtype=np.float32)
    Wr = np.asarray(W_real, dtype=np.float32)
    Wi = np.asarray(W_imag, dtype=np.float32)

    nc = _get_nc()
    R, Ri = _build_consts()
    G = _build_g(Wr, Wi)

    in_maps = []
    for c in range(NCORES):
        xc = x[c * BC:(c + 1) * BC]                       # [256, 16384]
        # -> [t, b, j] -> [pass, t(256), b(128), j(64)]
        xcp = xc.reshape(BC, KIN, BLOCK).transpose(2, 0, 1)
        xcp = xcp.reshape(BLOCK, NPASS, PB, KIN).transpose(1, 0, 2, 3)
        in_maps.append({
            "xP": np.ascontiguousarray(xcp).astype(NPBF16),
            "G": G, "R": R, "Ri": Ri,
        })
    res = run_bass_kernel_spmd(
        nc, in_maps, core_ids=list(range(NCORES)), trace=trace
    )
    out = np.concatenate([r["Y"] for r in res.results], axis=0)
    return np.ascontiguousarray(out.astype(np.float32)), res


def kernel(x, W_real, W_imag):
    out, _ = run(x, W_real, W_imag)
    return out



# revision 2
# speedup vs baseline: 1.0410x; 1.0410x over previous
"""Block-FFT circulant matmul (BlockFFTDirectPrior) as a Trainium2 Bass kernel.

Math: out = ifft( einsum('bjf,ijf->bif', fft(x_blocks), conj(W_full)) ).real
with 64x64 blocks of size 256, batch 2048.

All matmul-based (no FFT butterflies):
  stage 1: per input block j, spectrum = x_j @ R            (DFT as matmul)
  stage 2: per frequency slot s, mix blocks j -> i with a 128x128 real
           matrix G_s built from W_real/W_imag (re/im packed)
  stage 3: per output block i, time = spectrum_i @ Rinv     (IDFT as matmul)

v3 vs v2: the sync queue is reserved for the xbar transposes (the only
ops that block their queue for the full transfer) + the early pass-0
input load; all other DMA is moved to gpsimd/scalar.  Both stage-1
passes run back-to-back at the start (xk double-buffered via gpsimd
load) so perm1(p0) and perm1(p1) stream on sync underneath them; the
two perm2 xbars then ride under S2(p1) and S3(p0).  Five 32KB big
buffers are time-shared: A: xk(p0)->X2(p1), B: xk(p1)->O2(p1),
C: out1(p0)->O2(p0), D: out1(p1)->T2(p0), E: X2(p0)->T2(p1).

Sharding: data-parallel over batch across 8 NeuronCores (256 rows each),
2 passes of 128 rows per core.  All matmul operands are bf16.
"""

import numpy as np
import ml_dtypes

import concourse.bass as bass
import concourse.mybir as mybir
from concourse import bacc
from concourse.tile import TileContext
from concourse.bass_utils import run_bass_kernel_spmd

B, KIN, KOUT, BLOCK = 2048, 64, 64, 256
NCORES = 8
BC = B // NCORES            # 256 batch rows per core
NPASS = 2
PB = BC // NPASS            # 128 batch rows per pass

F32 = mybir.dt.float32
BF16 = mybir.dt.bfloat16
NPBF16 = ml_dtypes.bfloat16

_NC_CACHE = {}


def _build_consts():
    """DFT / inverse-DFT matrices, bf16, kernel layouts."""
    t = np.arange(BLOCK)
    f = np.arange(128)
    ang = 2.0 * np.pi * np.outer(t, f) / BLOCK          # [t, f]
    RA = np.cos(ang)                                    # re f=0..127
    RB = -np.sin(ang)                                   # im f=1..127
    RB[:, 0] = np.cos(np.pi * t)                        # re f=128 in col 0
    R = np.zeros((128, 4, 128), dtype=NPBF16)           # [t128, (h,kt), s]
    for h in range(2):
        M = RA if h == 0 else RB
        for kt in range(2):
            R[:, h * 2 + kt, :] = M[kt * 128:(kt + 1) * 128, :].astype(NPBF16)

    s = np.arange(128)
    tp = np.arange(BLOCK)
    angi = 2.0 * np.pi * np.outer(s, tp) / BLOCK        # [s, t']
    w = np.full((128, 1), 2.0 / BLOCK)
    w[0] = 1.0 / BLOCK
    RiA = w * np.cos(angi)
    RiB = -(2.0 / BLOCK) * np.sin(angi)
    RiB[0, :] = (1.0 / BLOCK) * np.cos(np.pi * tp)      # Nyquist (real) term
    Ri = np.concatenate([RiA, RiB], axis=1).astype(NPBF16)  # [128, 512]
    return R, Ri


def _build_g(Wr, Wi):
    """Stage-2 mixing matrices, layout [k=(h*64+j), s, m=(re_i|im_i)], bf16."""
    G = np.zeros((128, 128, 128), dtype=np.float32)     # [s, k, m]
    G[0, :64, :64] = Wr[:, :, 0].T
    G[0, 64:, 64:] = Wr[:, :, 128].T
    WrT = np.transpose(Wr, (2, 1, 0))                   # [f, j, i]
    WiT = np.transpose(Wi, (2, 1, 0))
    G[1:, :64, :64] = WrT[1:128]
    G[1:, :64, 64:] = -WiT[1:128]
    G[1:, 64:, :64] = WiT[1:128]
    G[1:, 64:, 64:] = WrT[1:128]
    return np.ascontiguousarray(G.transpose(1, 0, 2)).astype(NPBF16)


def _build_nc():
    nc = bacc.Bacc("TRN2", target_bir_lowering=False, debug=False)
    # xP layout [pass, t(256), b(128), j(64)]
    xP = nc.dram_tensor("xP", [NPASS, BLOCK, PB, KIN], BF16, kind="ExternalInput")
    Gt = nc.dram_tensor("G", [128, 128, 128], BF16, kind="ExternalInput")
    Rt = nc.dram_tensor("R", [128, 4 * 128], BF16, kind="ExternalInput")
    Rit = nc.dram_tensor("Ri", [128, 512], BF16, kind="ExternalInput")
    Y = nc.dram_tensor("Y", [BC, KOUT * BLOCK], BF16, kind="ExternalOutput")

    nck = [0]

    def copy_eng():
        nck[0] += 1
        return nc.vector.tensor_copy if nck[0] % 2 == 0 else nc.scalar.copy

    with TileContext(nc) as tc:
        with (
            tc.tile_pool(name="const", bufs=1) as cpool,
            tc.tile_pool(name="big", bufs=1) as bigpool,
            tc.tile_pool(name="work", bufs=2) as wpool,
            tc.tile_pool(name="ps", bufs=4, space="PSUM") as pspool,
        ):
            # constants: R/Ri on scalar queue (2 issues), G on gpsimd
            Rsb = cpool.tile([128, 4 * 128], BF16)
            nc.scalar.dma_start(Rsb[:, :], Rt.ap()[:, :])
            Risb = cpool.tile([128, 512], BF16)
            nc.scalar.dma_start(Risb[:, :], Rit.ap()[:, :])

            # five time-shared 32KB big buffers
            bufs = [
                bigpool.tile([128, 128 * 128], BF16, tag=f"buf{i}",
                             name=f"buf{i}")
                for i in range(5)
            ]
            # roles per pass: (xk, out1, X2, O2, T2)
            # A: xk0 -> X2(p1); B: xk1 -> O2(p1); C: out1(p0) -> O2(p0);
            # D: out1(p1) -> T2(p0); E: X2(p0) -> T2(p1)
            A, Bb, C, D, E = bufs
            xkr = [A, Bb]
            out1r = [C, D]
            X2r = [E, A]
            O2r = [C, Bb]
            T2r = [D, E]

            # xk: [t(128), kt, b(128), j(64)]
            # pass 0 chunks on sync (idle early); pass 1 on gpsimd.
            def load_xk(p, eng):
                xk = xkr[p].rearrange("p (kt b j) -> p kt b j", kt=2, j=KIN)
                for q in range(4):
                    for kt in range(2):
                        eng.dma_start(
                            xk[:, kt, q * 32:(q + 1) * 32, :],
                            xP.ap()[p, kt * 128:(kt + 1) * 128,
                                    q * 32:(q + 1) * 32],
                        )
                return xk

            # G load on gpsimd, s-ordered so stage 2 can consume
            # progressively (col-chunk c covers s = 16c..16c+16).
            def load_g():
                Gsb = cpool.tile([128, 128 * 128], BF16)
                for q in range(8):
                    nc.gpsimd.dma_start(
                        Gsb[:, q * 2048:(q + 1) * 2048],
                        Gt.ap()[:, q * 16:(q + 1) * 16],
                    )
                return Gsb

            def stage1(p, xk, nchunks=4):
                out1, X2 = out1r[p], X2r[p]
                out1v = out1.rearrange("p (b hj) -> p b hj", hj=128)
                X2v = X2.rearrange("p (b s) -> p b s", s=128)
                cstep = 8 // nchunks            # g2-groups per xbar chunk
                for g2 in range(8):             # 16-batch groups
                    for h in range(2):
                        ps = pspool.tile([128, 1024], F32, tag="ps")
                        for q in range(2):
                            g = g2 * 2 + q      # 8-batch chunk
                            nc.tensor.matmul(
                                ps[:, q * 512:(q + 1) * 512],
                                Rsb[:, (h * 2) * 128:(h * 2 + 1) * 128],
                                xk[:, 0, g * 8:(g + 1) * 8, :],
                                start=True, stop=False,
                            )
                            nc.tensor.matmul(
                                ps[:, q * 512:(q + 1) * 512],
                                Rsb[:, (h * 2 + 1) * 128:(h * 2 + 2) * 128],
                                xk[:, 1, g * 8:(g + 1) * 8, :],
                                start=False, stop=True,
                            )
                        # ps [s, (b16, j64)] -> out1 [s, b, h*64+j]
                        copy_eng()(
                            out1v[:, g2 * 16:(g2 + 1) * 16,
                                  h * 64:(h + 1) * 64],
                            ps.rearrange("p (b j) -> p b j", b=16),
                        )
                    # permute-1: fire an xbar chunk as soon as its
                    # b-range of out1 is complete
                    if g2 % cstep == cstep - 1:
                        c = g2 // cstep
                        nc.sync.dma_start(
                            X2v[:, c * cstep * 16:(c + 1) * cstep * 16, :],
                            out1[:, c * cstep * 2048:(c + 1) * cstep * 2048],
                            transpose=True,
                        )

            def stage2(p, Gsb):
                X2, O2 = X2r[p], O2r[p]
                X2v = X2.rearrange("p (b s) -> p b s", s=128)
                O2v = O2.rearrange("p (b s) -> p b s", s=128)
                for sg in range(16):            # 8 slots per PSUM tile
                    ps = pspool.tile([128, 1024], F32, tag="ps")
                    for u in range(8):
                        s = sg * 8 + u
                        nc.tensor.matmul(
                            ps[:, u * 128:(u + 1) * 128],
                            Gsb[:, s * 128:(s + 1) * 128],
                            X2v[:, :, s],
                            start=True, stop=True,
                        )
                    # ps [m, (s8, b)] -> O2 [m, b, s]  (strided read)
                    copy_eng()(
                        O2v[:, :, sg * 8:(sg + 1) * 8],
                        ps.rearrange("p (s b) -> p b s", s=8),
                    )

            def perm2(p):
                O2, T2 = O2r[p], T2r[p]
                T2v = T2.rearrange("p (b m) -> p b m", m=128)
                nc.sync.dma_start(T2v[:, :, :], O2[:, :], transpose=True)

            def stage3(p):
                T2 = T2r[p]
                T2v = T2.rearrange("p (b m) -> p b m", m=128)
                for g8 in range(8):             # 8 output blocks i per store
                    yt = wpool.tile([128, 2048], BF16, tag="yt", name="yt")
                    for half in range(2):
                        ps = pspool.tile([128, 1024], F32, tag="ps")
                        for q in range(4):
                            i = g8 * 8 + half * 4 + q
                            nc.tensor.matmul(
                                ps[:, q * 256:(q + 1) * 256],
                                T2v[:, :, i],
                                Risb[:, 0:256], start=True, stop=False,
                            )
                            nc.tensor.matmul(
                                ps[:, q * 256:(q + 1) * 256],
                                T2v[:, :, 64 + i],
                                Risb[:, 256:512], start=False, stop=True,
                            )
                        copy_eng()(
                            yt[:, half * 1024:(half + 1) * 1024], ps[:, :]
                        )
                    nc.gpsimd.dma_start(
                        Y.ap()[p * PB:(p + 1) * PB,
                               g8 * 2048:(g8 + 1) * 2048],
                        yt[:, :],
                    )

            # Pipelined emission.  PE: S1p0 S1p1 S2p0 S2p1 S3p0 S3p1.
            # sync: xk(p0) load, perm1(p0) chunks (under S1p0), perm1(p1)
            # chunks (under S1p1/S2p0), perm2(p0) (under S2p1), perm2(p1)
            # (under S3p0).
            xk0 = load_xk(0, nc.sync)
            xk1 = load_xk(1, nc.gpsimd)
            Gsb = load_g()
            stage1(0, xk0)
            stage1(1, xk1)
            stage2(0, Gsb)
            perm2(0)
            stage2(1, Gsb)
            perm2(1)
            stage3(0)
            stage3(1)
    nc.compile()
    return nc


def _get_nc():
    if "nc" not in _NC_CACHE:
        _NC_CACHE["nc"] = _build_nc()
    return _NC_CACHE["nc"]


def run(x, W_real, W_imag, trace=False):
    x = np.asarray(x, dtype=np.float32)
    Wr = np.asarray(W_real, dtype=np.float32)
    Wi = np.asarray(W_imag, dtype=np.float32)

    nc = _get_nc()
    R, Ri = _build_consts()
    G = _build_g(Wr, Wi)
    Rflat = np.ascontiguousarray(R.reshape(128, 512))

    in_maps = []
    for c in range(NCORES):
        xc = x[c * BC:(c + 1) * BC]                       # [256, 16384]
        # -> [t, b, j] -> [pass, t(256), b(128), j(64)]
        xcp = xc.reshape(BC, KIN, BLOCK).transpose(2, 0, 1)
        xcp = xcp.reshape(BLOCK, NPASS, PB, KIN).transpose(1, 0, 2, 3)
        in_maps.append({
            "xP": np.ascontiguousarray(xcp).astype(NPBF16),
            "G": G, "R": Rflat, "Ri": Ri,
        })
    res = run_bass_kernel_spmd(
        nc, in_maps, core_ids=list(range(NCORES)), trace=trace
    )
    out = np.concatenate([r["Y"] for r in res.results], axis=0)
    return np.ascontiguousarray(out.astype(np.float32)), res


def kernel(x, W_real, W_imag):
    out, _ = run(x, W_real, W_imag)
    return out


# revision 3
# speedup vs baseline: 1.0910x; 1.0480x over previous
"""Block-FFT circulant matmul (BlockFFTDirectPrior) as a Trainium2 Bass kernel.

Math: out = ifft( einsum('bjf,ijf->bif', fft(x_blocks), conj(W_full)) ).real
with 64x64 blocks of size 256, batch 2048.

All matmul-based (no FFT butterflies):
  stage 1: per input block j, spectrum = x_j @ R            (DFT as matmul)
  stage 2: per frequency slot s, mix blocks j -> i with a 128x128 real
           matrix G_s built from W_real/W_imag (re/im packed)
  stage 3: per output block i, time = spectrum_i @ Rinv     (IDFT as matmul)

v4: the DMA xbar transposes are gone.  Measurement showed DMA_TRANSPOSE
monopolizes the whole DMA fabric (~210 GB/s exclusive; concurrent
SWDGE/HWDGE wire traffic stalls to zero), making the kernel wire-bound.
The two inter-stage permutes are now done on the tensor engine
(is_transpose matmuls against an identity: 128x128 bf16 tiles,
SBUF -> PSUM bf16 -> copy evict), interleaved into the next stage's
matmul stream so HAM stays warm.  All DMA left is plain async traffic
(x in, G in, Y out) spread over sync/scalar/gpsimd queues.

Sharding: data-parallel over batch across 8 NeuronCores (256 rows each),
2 passes of 128 rows per core.  All matmul operands are bf16.
"""

import numpy as np
import ml_dtypes

import concourse.bass as bass
import concourse.mybir as mybir
from concourse import bacc
from concourse.tile import TileContext
from concourse.bass_utils import run_bass_kernel_spmd

B, KIN, KOUT, BLOCK = 2048, 64, 64, 256
NCORES = 8
BC = B // NCORES            # 256 batch rows per core
NPASS = 2
PB = BC // NPASS            # 128 batch rows per pass

F32 = mybir.dt.float32
BF16 = mybir.dt.bfloat16
NPBF16 = ml_dtypes.bfloat16

_NC_CACHE = {}


def _build_consts():
    """DFT / inverse-DFT matrices, bf16, kernel layouts."""
    t = np.arange(BLOCK)
    f = np.arange(128)
    ang = 2.0 * np.pi * np.outer(t, f) / BLOCK          # [t, f]
    RA = np.cos(ang)                                    # re f=0..127
    RB = -np.sin(ang)                                   # im f=1..127
    RB[:, 0] = np.cos(np.pi * t)                        # re f=128 in col 0
    R = np.zeros((128, 4, 128), dtype=NPBF16)           # [t128, (h,kt), s]
    for h in range(2):
        M = RA if h == 0 else RB
        for kt in range(2):
            R[:, h * 2 + kt, :] = M[kt * 128:(kt + 1) * 128, :].astype(NPBF16)

    s = np.arange(128)
    tp = np.arange(BLOCK)
    angi = 2.0 * np.pi * np.outer(s, tp) / BLOCK        # [s, t']
    w = np.full((128, 1), 2.0 / BLOCK)
    w[0] = 1.0 / BLOCK
    RiA = w * np.cos(angi)
    RiB = -(2.0 / BLOCK) * np.sin(angi)
    RiB[0, :] = (1.0 / BLOCK) * np.cos(np.pi * tp)      # Nyquist (real) term
    Ri = np.concatenate([RiA, RiB], axis=1).astype(NPBF16)  # [128, 512]
    return R, Ri


def _build_g(Wr, Wi):
    """Stage-2 mixing matrices, layout [k=(h*64+j), s, m=(re_i|im_i)], bf16."""
    G = np.zeros((128, 128, 128), dtype=np.float32)     # [s, k, m]
    G[0, :64, :64] = Wr[:, :, 0].T
    G[0, 64:, 64:] = Wr[:, :, 128].T
    WrT = np.transpose(Wr, (2, 1, 0))                   # [f, j, i]
    WiT = np.transpose(Wi, (2, 1, 0))
    G[1:, :64, :64] = WrT[1:128]
    G[1:, :64, 64:] = -WiT[1:128]
    G[1:, 64:, :64] = WiT[1:128]
    G[1:, 64:, 64:] = WrT[1:128]
    return np.ascontiguousarray(G.transpose(1, 0, 2)).astype(NPBF16)


def _build_nc():
    nc = bacc.Bacc("TRN2", target_bir_lowering=False, debug=False)
    # xP layout [pass, t(256), b(128), j(64)]
    xP = nc.dram_tensor("xP", [NPASS, BLOCK, PB, KIN], BF16, kind="ExternalInput")
    Gt = nc.dram_tensor("G", [128, 128, 128], BF16, kind="ExternalInput")
    Rt = nc.dram_tensor("R", [128, 4 * 128], BF16, kind="ExternalInput")
    Rit = nc.dram_tensor("Ri", [128, 512], BF16, kind="ExternalInput")
    It = nc.dram_tensor("Iden", [128, 128], BF16, kind="ExternalInput")
    Y = nc.dram_tensor("Y", [BC, KOUT * BLOCK], BF16, kind="ExternalOutput")

    nck = [0]

    def copy_eng():
        nck[0] += 1
        return nc.vector.tensor_copy if nck[0] % 2 == 0 else nc.scalar.copy

    with TileContext(nc) as tc:
        with (
            tc.tile_pool(name="const", bufs=1) as cpool,
            tc.tile_pool(name="big", bufs=1) as bigpool,
            tc.tile_pool(name="work", bufs=2) as wpool,
            tc.tile_pool(name="ps", bufs=3, space="PSUM") as pspool,
            tc.tile_pool(name="pt", bufs=2, space="PSUM") as ptpool,
        ):
            # constants: R/Ri/Iden on scalar queue, G on gpsimd
            Rsb = cpool.tile([128, 4 * 128], BF16)
            nc.scalar.dma_start(Rsb[:, :], Rt.ap()[:, :])
            Risb = cpool.tile([128, 512], BF16)
            nc.scalar.dma_start(Risb[:, :], Rit.ap()[:, :])
            Iden = cpool.tile([128, 128], BF16)
            nc.scalar.dma_start(Iden[:, :], It.ap()[:, :])

            # five time-shared 32KB big buffers
            bufs = [
                bigpool.tile([128, 128 * 128], BF16, tag=f"buf{i}",
                             name=f"buf{i}")
                for i in range(5)
            ]
            # roles: A: xk0 -> X2(p1); B: xk1 -> O2(p0); C: out1(p0) ->
            # O2(p1); D: out1(p1) -> T2(p0); E: X2(p0) -> T2(p1)
            A, Bb, C, D, E = bufs
            xkr = [A, Bb]
            out1r = [C, D]
            X2r = [E, A]
            O2r = [Bb, C]
            T2r = [D, E]

            # xk: [t(128), kt, b(128), j(64)]
            def load_xk(p, eng):
                xk = xkr[p].rearrange("p (kt b j) -> p kt b j", kt=2, j=KIN)
                for q in range(4):
                    for kt in range(2):
                        eng.dma_start(
                            xk[:, kt, q * 32:(q + 1) * 32, :],
                            xP.ap()[p, kt * 128:(kt + 1) * 128,
                                    q * 32:(q + 1) * 32],
                        )
                return xk

            # G load on gpsimd, s-ordered so stage 2 can consume
            # progressively (col-chunk c covers s = 16c..16c+16).
            def load_g():
                Gsb = cpool.tile([128, 128 * 128], BF16)
                for q in range(8):
                    nc.gpsimd.dma_start(
                        Gsb[:, q * 2048:(q + 1) * 2048],
                        Gt.ap()[:, q * 16:(q + 1) * 16],
                    )
                return Gsb

            # PE-transpose permute: src [p, b, c128] -> dst [c128, b, p],
            # emitted as a generator of (8-transpose + 1-evict) steps so
            # the caller can interleave steps into its own matmul stream.
            def perm_steps(src_v, dst_v):
                def step(b8):
                    pt = ptpool.tile([128, 1024], BF16, tag="pt")
                    for o in range(8):
                        b = b8 * 8 + o
                        nc.tensor.transpose(
                            pt[:, o * 128:(o + 1) * 128],
                            src_v[:, b, :], Iden[:, :],
                        )
                    copy_eng()(
                        dst_v[:, b8 * 8:(b8 + 1) * 8, :],
                        pt.rearrange("p (b c) -> p b c", b=8),
                    )
                return [lambda b8=b8: step(b8) for b8 in range(16)]

            def perm1_steps(p):
                out1v = out1r[p].rearrange("p (b hj) -> p b hj", hj=128)
                X2v = X2r[p].rearrange("p (b s) -> p b s", s=128)
                return perm_steps(out1v, X2v)

            def perm2_steps(p):
                O2v = O2r[p].rearrange("p (b s) -> p b s", s=128)
                T2v = T2r[p].rearrange("p (b m) -> p b m", m=128)
                return perm_steps(O2v, T2v)

            def stage1(p, xk, inter=()):
                out1 = out1r[p]
                out1v = out1.rearrange("p (b hj) -> p b hj", hj=128)
                it = iter(inter)
                for g2 in range(8):             # 16-batch groups
                    for h in range(2):
                        ps = pspool.tile([128, 1024], F32, tag="ps")
                        for q in range(2):
                            g = g2 * 2 + q      # 8-batch chunk
                            nc.tensor.matmul(
                                ps[:, q * 512:(q + 1) * 512],
                                Rsb[:, (h * 2) * 128:(h * 2 + 1) * 128],
                                xk[:, 0, g * 8:(g + 1) * 8, :],
                                start=True, stop=False,
                            )
                            nc.tensor.matmul(
                                ps[:, q * 512:(q + 1) * 512],
                                Rsb[:, (h * 2 + 1) * 128:(h * 2 + 2) * 128],
                                xk[:, 1, g * 8:(g + 1) * 8, :],
                                start=False, stop=True,
                            )
                        # ps [s, (b16, j64)] -> out1 [s, b, h*64+j]
                        copy_eng()(
                            out1v[:, g2 * 16:(g2 + 1) * 16,
                                  h * 64:(h + 1) * 64],
                            ps.rearrange("p (b j) -> p b j", b=16),
                        )
                        for fn in it:           # one interleaved perm step
                            fn()
                            break

            def stage2(p, Gsb, inter=()):
                X2, O2 = X2r[p], O2r[p]
                X2v = X2.rearrange("p (b s) -> p b s", s=128)
                O2v = O2.rearrange("p (b s) -> p b s", s=128)
                it = iter(inter)
                for sg in range(16):            # 8 slots per PSUM tile
                    ps = pspool.tile([128, 1024], F32, tag="ps")
                    for u in range(8):
                        s = sg * 8 + u
                        nc.tensor.matmul(
                            ps[:, u * 128:(u + 1) * 128],
                            Gsb[:, s * 128:(s + 1) * 128],
                            X2v[:, :, s],
                            start=True, stop=True,
                        )
                    # ps [m, (s8, b)] -> O2 [m, b, s]  (strided read)
                    copy_eng()(
                        O2v[:, :, sg * 8:(sg + 1) * 8],
                        ps.rearrange("p (s b) -> p b s", s=8),
                    )
                    for fn in it:
                        fn()
                        break

            def stage3(p, inter=()):
                T2 = T2r[p]
                T2v = T2.rearrange("p (b m) -> p b m", m=128)
                it = iter(inter)
                for g8 in range(8):             # 8 output blocks i per store
                    yt = wpool.tile([128, 2048], BF16, tag="yt", name="yt")
                    for half in range(2):
                        ps = pspool.tile([128, 1024], F32, tag="ps")
                        for q in range(4):
                            i = g8 * 8 + half * 4 + q
                            nc.tensor.matmul(
                                ps[:, q * 256:(q + 1) * 256],
                                T2v[:, :, i],
                                Risb[:, 0:256], start=True, stop=False,
                            )
                            nc.tensor.matmul(
                                ps[:, q * 256:(q + 1) * 256],
                                T2v[:, :, 64 + i],
                                Risb[:, 256:512], start=False, stop=True,
                            )
                        copy_eng()(
                            yt[:, half * 1024:(half + 1) * 1024], ps[:, :]
                        )
                        for fn in it:
                            fn()
                            break
                    nc.gpsimd.dma_start(
                        Y.ap()[p * PB:(p + 1) * PB,
                               g8 * 2048:(g8 + 1) * 2048],
                        yt[:, :],
                    )

            # Emission.  PE: S1p0, S1p1+T1p0, S2p0+T1p1, S2p1+T2p0,
            # S3p0+T2p1, S3p1.  DMA queues only carry plain transfers.
            xk0 = load_xk(0, nc.sync)
            xk1 = load_xk(1, nc.gpsimd)
            Gsb = load_g()
            stage1(0, xk0)
            stage1(1, xk1, inter=perm1_steps(0))
            stage2(0, Gsb, inter=perm1_steps(1))
            stage2(1, Gsb, inter=perm2_steps(0))
            stage3(0, inter=perm2_steps(1))
            stage3(1)
    nc.compile()
    return nc


def _get_nc():
    if "nc" not in _NC_CACHE:
        _NC_CACHE["nc"] = _build_nc()
    return _NC_CACHE["nc"]


def run(x, W_real, W_imag, trace=False):
    x = np.asarray(x, dtype=np.float32)
    Wr = np.asarray(W_real, dtype=np.float32)
    Wi = np.asarray(W_imag, dtype=np.float32)

    nc = _get_nc()
    R, Ri = _build_consts()
    G = _build_g(Wr, Wi)
    Rflat = np.ascontiguousarray(R.reshape(128, 512))
    Iden = np.eye(128, dtype=NPBF16)

    in_maps = []
    for c in range(NCORES):
        xc = x[c * BC:(c + 1) * BC]                       # [256, 16384]
        # -> [t, b, j] -> [pass, t(256), b(128), j(64)]
        xcp = xc.reshape(BC, KIN, BLOCK).transpose(2, 0, 1)
        xcp = xcp.reshape(BLOCK, NPASS, PB, KIN).transpose(1, 0, 2, 3)
        in_maps.append({
            "xP": np.ascontiguousarray(xcp).astype(NPBF16),
            "G": G, "R": Rflat, "Ri": Ri, "Iden": Iden,
        })
    res = run_bass_kernel_spmd(
        nc, in_maps, core_ids=list(range(NCORES)), trace=trace
    )
    out = np.concatenate([r["Y"] for r in res.results], axis=0)
    return np.ascontiguousarray(out.astype(np.float32)), res


def kernel(x, W_real, W_imag):
    out, _ = run(x, W_real, W_imag)
    return out
